# revision 1
# baseline (speedup 1.0000x reference)
"""Trainium2 Bass kernel for nn_AudioToJoints (single-layer LSTM + FC).

Strategy (8 NeuronCores, one chip):
- Shard the 4H gate dimension 8 ways: logical core j owns hidden chunk
  j*128:(j+1)*128, i.e. the four 128-row gate blocks (f,i,g,o) for that chunk.
- Per timestep, each core computes its 512 gates = W_own @ [h_full; x_t] (bf16
  matmuls, fp32 PSUM), applies the LSTM pointwise ops for its chunk (fp32
  c-state), and broadcasts its new h chunk (bf16, [128,32]) to all 8 cores via
  remote_dma_broadcast into slot <logical id> of a parity-alternating gather
  buffer.  No collectives; sync is remote semaphores with monotonic
  register thresholds inside one raw bass Fori loop.
- Every step the full gathered h ([128, 8*32] bf16) is DMA'd to DRAM; after the
  loop each core runs the FC projection over its own T/8 slice of timesteps
  and writes predT [51, (T/8)*32] fp32.  Host reassembles [B, T, O].
"""
import sys

sys.path.insert(0, "/opt/trn_rl_repo")

import numpy as np
import ml_dtypes

from concourse import bass, bacc, mybir

B, T_FULL, I, H, O = 32, 2048, 128, 1024, 51
NC = 8
HC = H // NC          # 128 hidden dims per core
KT = NC + 1           # 9 contraction k-tiles: 8 h chunks + 1 x
MT = 4                # 4 gate blocks (f, i, g, o)
AF = mybir.ActivationFunctionType
f32 = mybir.dt.float32
bf16 = mybir.dt.bfloat16
u32 = mybir.dt.uint32
RD = [(0, k) for k in range(8)]          # barrier: all 8 peers incl self
RDX = [None] + [(0, k) for k in range(1, 8)]  # data: exclude self


def build(T: int, self_bcast: bool = True, do_fc: bool = True, do_store: bool = True,
          repeats: int = 1):
    assert T % (2 * NC) == 0
    TS = T // NC              # timesteps per core for FC
    NG = (TS * B) // 512      # fc column groups of 512
    assert (TS * B) % 512 == 0

    nc = bacc.Bacc("TRN2", target_bir_lowering=False, debug=False, num_devices=NC,
                   detect_race_conditions=False)

    # ---------------- DRAM ----------------
    wk_d = nc.dram_tensor("wk", [128, KT * MT * 128], bf16, kind="ExternalInput")
    bias_d = nc.dram_tensor("bias", [128, MT], f32, kind="ExternalInput")
    xT_d = nc.dram_tensor("xT", [(T + 2) * 128, B], bf16, kind="ExternalInput")
    wfc_d = nc.dram_tensor("wfc", [128, NC * O], bf16, kind="ExternalInput")

    hsd_d = nc.dram_tensor("hsd", [(T + 1) * 128, NC * B], bf16)  # h(0)..h(T)
    pred_d = nc.dram_tensor("predT", [O, TS * B], f32, kind="ExternalOutput")
    myid_d = nc.dram_tensor("myid", [1, 1], u32, kind="ExternalOutput")

    # ---------------- SBUF ----------------
    w_sb = nc.alloc_sbuf_tensor("w_sb", [128, KT * MT * 128], bf16)
    bias_sb = nc.alloc_sbuf_tensor("bias_sb", [128, MT], f32)
    xbuf = nc.alloc_sbuf_tensor("xbuf", [128, 2 * B], bf16)
    gather = nc.alloc_sbuf_tensor("gather", [128, 2 * NC * B], bf16)
    hstage = nc.alloc_sbuf_tensor("hstage", [128, 2 * B], bf16)
    c_sb = nc.alloc_sbuf_tensor("c_sb", [128, B], f32)
    tf = nc.alloc_sbuf_tensor("tf", [128, 2 * B], f32)
    ti = nc.alloc_sbuf_tensor("ti", [128, 2 * B], f32)
    tg = nc.alloc_sbuf_tensor("tg", [128, 2 * B], f32)
    to = nc.alloc_sbuf_tensor("to", [128, 2 * B], f32)
    ttc = nc.alloc_sbuf_tensor("ttc", [128, 2 * B], f32)
    tig = nc.alloc_sbuf_tensor("tig", [128, 2 * B], f32)
    wfc_sb = nc.alloc_sbuf_tensor("wfc_sb", [128, NC * O], bf16)
    fcmov = nc.alloc_sbuf_tensor("fcmov", [128, 2 * NC * 512], bf16)
    fcout = nc.alloc_sbuf_tensor("fcout", [O, 2 * 512], f32)
    idt = nc.alloc_sbuf_tensor("idt", [1, 1], u32)

    # ---------------- PSUM: 8 banks = 4 gates x 2 parities ----------------
    pp = [[nc.alloc_psum_tensor(f"ps_{m}_{p}", [128, 512], f32) for p in range(2)]
          for m in range(MT)]

    # ---------------- semaphores ----------------
    wsem = nc.alloc_semaphore("wsem")    # weight/bias/wfc/myid loads
    msem = nc.alloc_semaphore("msem")    # memsets
    xsem = nc.alloc_semaphore("xsem")    # x tile loads (+16 each)
    rsem = nc.alloc_semaphore("rsem")    # remote h arrivals (+2 x 7 peers per step)
    scp = nc.alloc_semaphore("scp")      # own gather slot written (+1 per step)
    lsem = nc.alloc_semaphore("lsem")    # broadcast send-complete (+16)
    psem = nc.alloc_semaphore("psem")    # broadcast desc written (+1)
    hsem = nc.alloc_semaphore("hsem")    # h chunk ready (+1 per step)
    barsem = nc.alloc_semaphore("barsem")  # startup barrier
    smm = nc.alloc_semaphore("smm")      # PE m-tile done (+4 per step)
    sact = nc.alloc_semaphore("sact")    # ACT ops (+5 per step)
    sdve = nc.alloc_semaphore("sdve")    # DVE ops (+3 per step)
    stsem = nc.alloc_semaphore("stsem")  # hsd stores (+16 per step)
    fcd = nc.alloc_semaphore("fcd")      # fc moving loads
    fcpe = nc.alloc_semaphore("fcpe")    # fc matmul groups
    fca = nc.alloc_semaphore("fca")      # fc psum evacuations
    fco = nc.alloc_semaphore("fco")      # fc output stores

    RINC = 16 if self_bcast else 14
    pe, act, dve, gp = nc.tensor, nc.scalar, nc.vector, nc.gpsimd

    # ---------------- setup ----------------
    gp.dma_start(out=w_sb[:], in_=wk_d[:]).then_inc(wsem, 16)
    gp.dma_start(out=bias_sb[:], in_=bias_d[:]).then_inc(wsem, 16)
    gp.dma_start(out=wfc_sb[:], in_=wfc_d[:]).then_inc(wsem, 16)
    gp.dma_start(out=idt[:], in_=nc.partition_id_tensor[:]).then_inc(wsem, 16)
    nc.sync.dma_start(out=xbuf[:, 0:B], in_=xT_d[0:128, :]).then_inc(xsem, 16)
    nc.sync.dma_start(out=xbuf[:, B:2 * B], in_=xT_d[128:256, :]).then_inc(xsem, 16)

    dve.memset(c_sb[:], 0.0).then_inc(msem, 1)
    dve.memset(gather[:], 0.0).then_inc(msem, 1)

    pid = gp.partition_id()
    pid_all = nc.partition_id() if not self_bcast else None

    # startup barrier: every core's memsets done before any remote write lands
    gp.wait_ge(msem, 2)
    gp.wait_ge(wsem, 64)
    gp.dma_start(out=myid_d[:], in_=idt[:]).then_inc(wsem, 16)
    gp.remote_sem_update_broadcast(remote_sem=barsem, local_sem=lsem, rdests=RD
                                   ).then_inc(psem, 1)
    gp.wait_ge(psem, 1)
    gp.trigger_dma(count=1)
    gp.wait_ge(barsem, 16)

    pe.wait_ge(wsem, 64)
    pe.wait_ge(msem, 2)
    pe.wait_ge(barsem, 16)
    act.wait_ge(wsem, 64)
    dve.wait_ge(barsem, 16)
    nc.sync.wait_ge(msem, 2)

    # threshold registers
    r_px = pe.alloc_register("r_px"); pe.reg_mov(r_px, 16)
    r_pc = pe.alloc_register("r_pc"); pe.reg_mov(r_pc, 0)
    r_pr = pe.alloc_register("r_pr"); pe.reg_mov(r_pr, 0)
    r_amm = act.alloc_register("r_amm"); act.reg_mov(r_amm, 0)
    r_adve = act.alloc_register("r_adve"); act.reg_mov(r_adve, 3)
    r_da = dve.alloc_register("r_da"); dve.reg_mov(r_da, 0)
    r_dl = dve.alloc_register("r_dl"); dve.reg_mov(r_dl, 16)
    r_dd1 = dve.alloc_register("r_dd1"); dve.reg_mov(r_dd1, 2)
    r_dd2 = dve.alloc_register("r_dd2"); dve.reg_mov(r_dd2, 0)
    r_gh = gp.alloc_register("r_gh"); gp.reg_mov(r_gh, 0)
    r_gp = gp.alloc_register("r_gp"); gp.reg_mov(r_gp, 1)
    r_gr = gp.alloc_register("r_gr"); gp.reg_mov(r_gr, 0)
    r_gs = gp.alloc_register("r_gs"); gp.reg_mov(r_gs, 0)
    r_gc = gp.alloc_register("r_gc"); gp.reg_mov(r_gc, 0)
    r_gx = nc.sync.alloc_register("r_gx"); nc.sync.reg_mov(r_gx, 32)
    r_sh = nc.sync.alloc_register("r_sh"); nc.sync.reg_mov(r_sh, 0)
    r_sr = nc.sync.alloc_register("r_sr"); nc.sync.reg_mov(r_sr, 0)
    r_sc = nc.sync.alloc_register("r_sc"); nc.sync.reg_mov(r_sc, 0)

    def step(pos, q):
        """Emit one timestep; t = 2*q + pos, parity p = pos."""
        p = pos
        hp = 1 - pos  # parity of h(t+1) / next gather buffer

        # ---- PE: x k-tile first (overlaps broadcast delivery), then
        # rsem wait, then the 8 h-slot k-tiles per gate bank ----
        pe.reg_add(r_px, r_px, 16)
        pe.wait_ge(xsem, r_px)
        for m in range(MT):
            kx = NC * MT + m
            pe.matmul(pp[m][p][:, 0:B],
                      w_sb[:, kx * 128:(kx + 1) * 128],
                      xbuf[:, p * B:(p + 1) * B],
                      start=True, stop=False, skip_group_check=True)
        pe.wait_ge(rsem, r_pr)
        pe.reg_add(r_pr, r_pr, RINC)
        if not self_bcast:
            pe.wait_ge(scp, r_pc)
            pe.reg_add(r_pc, r_pc, 1)
        for m in range(MT):
            mm = None
            for k in range(NC):
                lhsT = w_sb[:, (k * MT + m) * 128:(k * MT + m + 1) * 128]
                rhs = gather[:, (p * NC + k) * B:(p * NC + k + 1) * B]
                mm = pe.matmul(pp[m][p][:, 0:B], lhsT, rhs,
                               start=False, stop=(k == NC - 1),
                               skip_group_check=True)
            mm.then_inc(smm, 1)

        # ---- ACT: nonlinearities (order f, i, g, o, tanh_c) ----
        for m, (buf, fn) in enumerate([(tf, AF.Sigmoid), (ti, AF.Sigmoid),
                                       (tg, AF.Tanh), (to, AF.Sigmoid)]):
            act.reg_add(r_amm, r_amm, 1)
            act.wait_ge(smm, r_amm)
            act.activation(buf[:, p * B:(p + 1) * B], pp[m][p][:, 0:B], fn,
                           bias=bias_sb[:, m:m + 1]).then_inc(sact, 1)
        act.wait_ge(sdve, r_adve)
        act.reg_add(r_adve, r_adve, 3)
        act.activation(ttc[:, p * B:(p + 1) * B], c_sb[:], AF.Tanh
                       ).then_inc(sact, 1)

        # ---- DVE: c and h updates (every op carries a real sem inc) ----
        sl = slice(p * B, (p + 1) * B)
        dve.reg_add(r_da, r_da, 1)
        dve.wait_ge(sact, r_da)
        dve.wait_ge(sdve, r_dd2)        # cadd(t-1) visible before overwriting c
        dve.reg_add(r_dd2, r_dd2, 3)
        dve.tensor_mul(c_sb[:], tf[:, sl], c_sb[:]).then_inc(sdve, 1)
        dve.reg_add(r_da, r_da, 2)
        dve.wait_ge(sact, r_da)
        dve.tensor_mul(tig[:, sl], ti[:, sl], tg[:, sl]).then_inc(sdve, 1)
        dve.wait_ge(sdve, r_dd1)        # cmul+tigmul visible
        dve.reg_add(r_dd1, r_dd1, 3)
        dve.tensor_add(c_sb[:], c_sb[:], tig[:, sl]).then_inc(sdve, 1)
        dve.reg_add(r_da, r_da, 2)
        dve.wait_ge(sact, r_da)
        dve.wait_ge(lsem, r_dl)
        dve.reg_add(r_dl, r_dl, 16)
        dve.tensor_mul(hstage[:, hp * B:(hp + 1) * B], to[:, sl], ttc[:, sl]
                       ).then_inc(hsem, 1)
        if not self_bcast:
            dve.tensor_mul(gather[:, bass.ds(pid_all * B + hp * NC * B, B)],
                           to[:, sl], ttc[:, sl]).then_inc(scp, 1)

        # ---- GP: hsd store, broadcast h(t+1), x prefetch ----
        for k8 in range(NC):
            with gp.If(pid == k8):
                gp.remote_dma_broadcast(
                    out_ap=gather[:, (hp * NC + k8) * B:(hp * NC + k8 + 1) * B],
                    in_ap=hstage[:, hp * B:(hp + 1) * B],
                    remote_sem=rsem, local_sem=lsem,
                    rdests=(RD if self_bcast else RDX),
                ).then_inc(psem, 1)
            gp.end_ifs()
        # store gathered h(t) to DRAM (before trigger: peers' h(t+2) writes
        # into gather[p] are transitively gated on our trigger(t+1))
        nc.sync.wait_ge(rsem, r_sr)
        nc.sync.reg_add(r_sr, r_sr, RINC)
        if not self_bcast:
            nc.sync.reg_add(r_sc, r_sc, 1)
            nc.sync.wait_ge(scp, r_sc)
        if do_store:
            t_sv = q * 2 + pos
            nc.sync.dma_start(out=hsd_d[bass.ds(t_sv * 128, 128), :],
                              in_=gather[:, p * NC * B:(p + 1) * NC * B]
                              ).then_inc(stsem, 16)
        gp.reg_add(r_gh, r_gh, 1)
        gp.wait_ge(hsem, r_gh)
        gp.reg_add(r_gp, r_gp, 1)
        gp.wait_ge(psem, r_gp)
        if do_store:
            gp.wait_ge(stsem, r_gs)     # store(t-1) done (2-step slack vs peers)
            gp.reg_add(r_gs, r_gs, 16)
        gp.trigger_dma(count=1)
        # x(t+2) into same parity slot (sync engine; serialize loads)
        t2 = q * 2 + (pos + 2)
        nc.sync.reg_add(r_sh, r_sh, 1)
        nc.sync.wait_ge(hsem, r_sh)
        nc.sync.wait_ge(xsem, r_gx)
        nc.sync.reg_add(r_gx, r_gx, 16)
        nc.sync.dma_start(out=xbuf[:, p * B:(p + 1) * B],
                          in_=xT_d[bass.ds(t2 * 128, 128), :]).then_inc(xsem, 16)

    if repeats == 1:
        with nc.Fori(0, T // 2) as q:
            step(0, q)
            step(1, q)
    else:
        with nc.Fori(0, repeats):
            with nc.Fori(0, T // 2) as q:
                step(0, q)
                step(1, q)

    TT = repeats * T  # total steps executed (for drain thresholds)
    if do_fc and do_store:
        # ---------------- final store of h(T), then FC tail ----------------
        nc.sync.wait_ge(rsem, r_sr)
        if not self_bcast:
            nc.sync.wait_ge(scp, r_sc)
        nc.sync.dma_start(out=hsd_d[T * 128:(T + 1) * 128, :],
                          in_=gather[:, 0:NC * B]).then_inc(stsem, 16)
        lo = pid * TS  # first timestep of my slice
        gp.wait_ge(stsem, 16 * (TT + 1))
        gp.wait_ge(hsem, TT)  # everything drained on gp before reusing queues

        hsd3 = hsd_d  # [(t*128 + hd), slot*32+b]
        for g in range(NG):
            # moving tiles: 16 timesteps x 32 batch for each chunk s
            if g >= 2:
                gp.wait_ge(fcpe, g - 1)
            for s in range(NC):
                buf = fcmov[:, ((g % 2) * NC + s) * 512:((g % 2) * NC + s + 1) * 512]
                src = hsd3[bass.ds((lo + (g * 16 + 1)) * 128, 16 * 128),
                           s * B:(s + 1) * B]
                src = src.rearrange("(t p) b -> p t b", p=128)
                gp.dma_start(out=buf, in_=src).then_inc(fcd, 16)
            if g >= 1:
                pe.wait_ge(fca, g - 1)
            pe.wait_ge(fcd, 16 * NC * (g + 1))
            mm = None
            for s in range(NC):
                buf = fcmov[:, ((g % 2) * NC + s) * 512:((g % 2) * NC + s + 1) * 512]
                mm = pe.matmul(pp[0][g % 2][0:O, 0:512],
                               wfc_sb[:, s * O:(s + 1) * O], buf,
                               start=(s == 0), stop=(s == NC - 1))
            mm.then_inc(fcpe, 1)
            act.wait_ge(fcpe, g + 1)
            if g >= 2:
                act.wait_ge(fco, 16 * (g - 1))
            act.copy(fcout[0:O, (g % 2) * 512:(g % 2 + 1) * 512],
                     pp[0][g % 2][0:O, 0:512]).then_inc(fca, 1)
            gp.wait_ge(fca, g + 1)
            gp.dma_start(out=pred_d[:, g * 512:(g + 1) * 512],
                         in_=fcout[0:O, (g % 2) * 512:(g % 2 + 1) * 512]
                         ).then_inc(fco, 16)
        gp.wait_ge(fco, 16 * NG)
        gp.wait_ge(wsem, 80)


    nc.sync.wait_ge(xsem, 16 * (repeats * T + 2))
    nc.finalize()
    return nc


# ---------------------------------------------------------------------------
# host side
# ---------------------------------------------------------------------------

def _to_bf16(a):
    return np.asarray(a).astype(ml_dtypes.bfloat16)


def prepare_inputs(x, W_ih, W_hh, b_ih, b_hh, W_fc, b_fc, T):
    """Build per-core in_maps (logical core j -> list index j)."""
    Wcat = np.concatenate([np.asarray(W_hh), np.asarray(W_ih)], axis=1)  # [4H, H+I]
    bias = np.asarray(b_ih + b_hh, np.float32)
    # x -> [T+2, I, B] -> [(T+2)*128, B]
    xT = np.zeros((T + 2, I, B), np.float32)
    xT[:T] = np.asarray(x)[:, :T, :].transpose(1, 2, 0)
    xT2 = _to_bf16(xT.reshape((T + 2) * I, B))

    gate_rows = lambda blk, j: slice(blk * H + j * HC, blk * H + (j + 1) * HC)
    BLK = [1, 0, 2, 3]  # m order f, i, g, o -> torch blocks i(0) f(1) g(2) o(3)

    in_maps = []
    for j in range(NC):
        wk = np.zeros((128, KT * MT * 128), np.float32)
        bias_m = np.zeros((128, MT), np.float32)
        for m in range(MT):
            rows = Wcat[gate_rows(BLK[m], j)]            # [128, H+I]
            bias_m[:, m] = bias[gate_rows(BLK[m], j)]
            for k in range(KT):
                tile = rows[:, k * 128:(k + 1) * 128].T  # [128 contract, 128 gate]
                wk[:, (k * MT + m) * 128:(k * MT + m + 1) * 128] = tile
        wfc = np.zeros((128, NC * O), np.float32)
        for s in range(NC):
            wfc[:, s * O:(s + 1) * O] = np.asarray(W_fc)[:, s * HC:(s + 1) * HC].T
        in_maps.append({
            "wk": _to_bf16(wk),
            "bias": bias_m,
            "xT": xT2,
            "wfc": _to_bf16(wfc),
        })
    return in_maps


def assemble_output(results, b_fc, T):
    TS = T // NC
    pred = np.empty((B, T, O), np.float32)
    for j in range(NC):
        sl = results[j]["predT"]                       # [O, TS*B]
        blk = sl.reshape(O, TS, B).transpose(2, 1, 0)  # [B, TS, O]
        pred[:, j * TS:(j + 1) * TS, :] = blk
    if np.any(np.asarray(b_fc) != 0):
        pred = pred + np.asarray(b_fc, np.float32)
    return pred


_NC_CACHE = {}


def kernel(x, W_ih, W_hh, b_ih, b_hh, W_fc, b_fc):
    from concourse.bass_utils import run_bass_kernel_spmd
    T = x.shape[1]
    if T not in _NC_CACHE:
        _NC_CACHE[T] = build(T)
    nc = _NC_CACHE[T]
    in_maps = prepare_inputs(x, W_ih, W_hh, b_ih, b_hh, W_fc, b_fc, T)
    res = run_bass_kernel_spmd(nc, in_maps, list(range(NC)))
    return assemble_output(res.results, b_fc, T)

